# revision 1
# baseline (speedup 1.0000x reference)
"""Multi-head causal attention (B=2, T=2048, D=1024, H=16, Dh=64) on 8 trn2 cores.

Sharding: head-parallel. Core c computes heads (2c, 2c+1) for both batch rows:
  - QKV projections for its 128-dim head slice (fp32r matmuls, K=1024)
  - causal attention for its 2 heads x 2 batches (no max-subtraction softmax;
    scores are O(5) so exp() is safe in fp32; 1/sqrt(Dh) folded into Wq)
  - partial output projection out_c = ctx_c @ Wo.T[slice]  -> [1024, 4096]
Host sums the 8 partials, adds bias, reshapes.

All matmuls run as float32r (tf32-class, ~1.5e-4 rel err, 4x faster than fp32).
Scores are computed transposed (ST[tk, tq]) so no P-matrix transposes are needed:
softmax renormalization works by appending 64 replicated ones-columns to V, so the
denominator Z lands replicated in ctx partitions 64-127 and 1/Z = exp(-ln Z) is
computed partition-parallel on the scalar engine straight from PSUM.
"""

import os
import sys

for _p in ("/opt/trn_rl_repo", "/opt/pypackages",
           "/root/.axon_site/_ro/trn_rl_repo", "/root/.axon_site/_ro/pypackages"):
    if os.path.isdir(_p) and _p not in sys.path:
        sys.path.append(_p)

import numpy as np
import concourse.bass as bass  # noqa: F401  (engine classes referenced via nc)
import concourse.tile as tile
from concourse import bacc, mybir
from concourse.bass_utils import run_bass_kernel_spmd
import concourse.bass_utils as _bu

if os.environ.get("LDW_OPT", "1") == "1" and not getattr(_bu, "_ldw_patched", False):
    _orig_run_command = _bu.run_command

    def _patched_run_command(argv, **kwargs):
        argv = [a.replace("--enable-ldw-opt=false", "--enable-ldw-opt=true")
                if isinstance(a, str) else a for a in argv]
        return _orig_run_command(argv, **kwargs)

    _bu.run_command = _patched_run_command
    _bu._ldw_patched = True

F32 = mybir.dt.float32
F32R = mybir.dt.float32r
AF = mybir.ActivationFunctionType

B, T, D = 2, 2048, 1024
H, DH = 16, 64
NTOK = B * T          # 4096
NCORES = 8
HPC = H // NCORES     # heads per core = 2
DSL = HPC * DH        # per-core d-slice width = 128
KT = D // 128         # contraction tiles = 8
NBLK = T // 512       # tq blocks per batch = 4
NTKT = T // 128       # tk tiles per batch = 16


def _build_nc():
    nc = bacc.Bacc("TRN2", target_bir_lowering=False, debug=False)

    xT = nc.dram_tensor("xT", [D, NTOK], F32R, kind="ExternalInput").ap()
    wq = nc.dram_tensor("wq", [D, DSL], F32R, kind="ExternalInput").ap()
    wk = nc.dram_tensor("wk", [D, DSL], F32R, kind="ExternalInput").ap()
    wv = nc.dram_tensor("wv", [D, DSL], F32R, kind="ExternalInput").ap()
    wo = nc.dram_tensor("wo", [DSL, D], F32R, kind="ExternalInput").ap()
    mask = nc.dram_tensor("mask", [128, 256], F32, kind="ExternalInput").ap()
    ident = nc.dram_tensor("ident", [128, 128], F32, kind="ExternalInput").ap()
    outp = nc.dram_tensor("outp", [D, NTOK], F32, kind="ExternalOutput").ap()

    with tile.TileContext(nc) as tc:
        _emit(nc, tc, xT, wq, wk, wv, wo, mask, ident, outp)
    nc.compile()
    return nc


def _emit(nc, tc, xT, wq, wk, wv, wo, mask, ident, outp):
    from contextlib import ExitStack

    ctx = ExitStack()
    const = ctx.enter_context(tc.tile_pool(name="const", bufs=1))
    sb = ctx.enter_context(tc.tile_pool(name="sb", bufs=2))
    pt_pool = ctx.enter_context(tc.tile_pool(name="ptp", bufs=4))
    ob_pool = ctx.enter_context(tc.tile_pool(name="obp", bufs=6))
    ps = ctx.enter_context(tc.tile_pool(name="ps", bufs=1, space="PSUM"))

    # ---- constants ----
    wq_sb = const.tile([128, KT, DSL], F32R)
    wk_sb = const.tile([128, KT, DSL], F32R)
    wv_sb = const.tile([128, KT, DSL], F32R)
    nc.sync.dma_start(wq_sb[:], wq.rearrange("(k p) m -> p k m", p=128))
    nc.sync.dma_start(wk_sb[:], wk.rearrange("(k p) m -> p k m", p=128))
    nc.sync.dma_start(wv_sb[:], wv.rearrange("(k p) m -> p k m", p=128))

    wo_sb = const.tile([DSL, D], F32R)
    mask_sb = const.tile([128, 256], F32)
    ident_sb = const.tile([128, 128], F32)
    onecol_f = const.tile([128, 1], F32)
    nc.vector.memset(onecol_f[:], 1.0)


    xTr = xT.rearrange("(k p) t -> p k t", p=128)  # [128, 8, 4096]

    qT, kTt, v_ext = {}, {}, {}
    for b in range(B):
        qT[b] = sb.tile([128, T], F32R, tag="qT", name=f"qT{b}")
        kTt[b] = sb.tile([128, T], F32R, tag="kT", name=f"kT{b}")
        v_ext[b] = sb.tile([128, NTKT, 4 * DH], F32R, tag="vext", name=f"vext{b}")
        vons = v_ext[b][:].rearrange("p k (h c) -> p (k h) c", c=2 * DH)[:, :, DH : 2 * DH]
        nc.vector.tensor_copy(vons, onecol_f[:, 0:1].to_broadcast((128, 2 * NTKT, DH)))

    xblk_t = {}

    def emit_xdma(b, blk):
        t0 = b * T + blk * 512
        xblk_t[(b, blk)] = sb.tile([128, KT, 512], F32R, tag="xblk", name=f"xblk{b}_{blk}")
        for k in range(KT):
            nc.sync.dma_start(xblk_t[(b, blk)][:, k, :], xTr[:, k, t0 : t0 + 512])

    def emit_qkv(b, blk):
        xblk = xblk_t.pop((b, blk))
        for wname, w_sb in (("q", wq_sb), ("k", wk_sb), ("v", wv_sb)):
            pp = ps.tile([128, 512], F32, tag="mm", name=f"pp{wname}{b}_{blk}", bufs=2)
            for k in range(KT):
                nc.tensor.matmul(
                    pp[:], w_sb[:, k, :], xblk[:, k, :],
                    start=(k == 0), stop=(k == KT - 1),
                )
            if wname == "q":
                nc.vector.tensor_copy(qT[b][:, blk * 512 : (blk + 1) * 512], pp[:])
            elif wname == "k":
                nc.vector.tensor_copy(kTt[b][:, blk * 512 : (blk + 1) * 512], pp[:])
            else:
                vst = sb.tile([128, 512], F32, tag="vst", name=f"vst{b}_{blk}")
                nc.scalar.copy(vst[:], pp[:])
                tr4 = ps.tile([128, 512], F32, tag="mm", name=f"tr4{b}_{blk}", bufs=2)
                for j in range(4):
                    nc.tensor.transpose(tr4[:, j * 128 : (j + 1) * 128],
                                        vst[:, j * 128 : (j + 1) * 128], ident_sb[:])
                dst = v_ext[b][:, blk * 4 : (blk + 1) * 4, :].rearrange(
                    "p j (h c) -> p j h c", c=2 * DH)[:, :, :, 0:DH]
                nc.vector.tensor_copy(dst, tr4[:].rearrange("p (j h c) -> p j h c", j=4, c=DH))

    def emit_attn(b, qi):
        tb = b * T
        q0 = qi * 512
        ntk = 4 * qi + 4
        ctx_pair = ps.tile([128, 2, 512], F32, tag="ctx", name=f"ctx_{b}_{qi}")
        for tk in range(ntk):
            r = tk - 4 * qi
            c0 = 0 if r < 0 else min(128 * r, 256)
            sp = ps.tile([128, 2, 512], F32, tag="s", name=f"sp{b}_{qi}_{tk}", bufs=2)
            for h in range(2):
                hs = slice(h * DH, (h + 1) * DH)
                nc.tensor.matmul(
                    sp[:, h, c0:512],
                    kTt[b][hs, tk * 128 : (tk + 1) * 128],
                    qT[b][hs, q0 + c0 : q0 + 512],
                    start=True, stop=True,
                )
            pt = pt_pool.tile([128, 2, 512], F32R, tag="pt", name=f"pt{b}_{qi}_{tk}")
            nc.scalar.activation(pt[:, :, c0:512], sp[:, :, c0:512], AF.Exp)
            if r >= 0:
                mL = 256 if r == 3 else 128
                msl = mask_sb[:, 256 - mL : 256]
                for h in range(2):
                    seg = pt[:, h, c0 : c0 + mL]
                    nc.vector.tensor_mul(seg, seg, msl)
            for h in range(2):
                nc.tensor.matmul(
                    ctx_pair[:, h, c0:512],
                    v_ext[b][:, tk, h * 2 * DH : (h + 1) * 2 * DH],
                    pt[:, h, c0:512],
                    start=(tk == 0), stop=(tk == ntk - 1),
                )
        lnz = sb.tile([DH, 2, 512], F32, tag="lnz", name=f"lnz_{b}_{qi}")
        nc.scalar.activation(lnz[:], ctx_pair[DH:128, :, :], AF.Ln)
        rz = sb.tile([DH, 2, 512], F32, tag="rz", name=f"rz_{b}_{qi}")
        nc.scalar.activation(rz[:], lnz[:], AF.Exp, scale=-1.0)
        cn = sb.tile([128, 512], F32R, tag="cn", name=f"cn_{b}_{qi}", bufs=3)
        for h in range(2):
            nc.vector.tensor_mul(cn[h * DH : (h + 1) * DH, :],
                                 ctx_pair[0:DH, h, :], rz[:, h, :])
        for od0 in range(0, 8, 2):
            ob2 = ob_pool.tile([128, 2, 512], F32, tag="ob", name=f"ob{b}_{qi}_{od0}")
            for j in range(2):
                od = od0 + j
                op = ps.tile([128, 512], F32, tag="mm", name=f"op{b}_{qi}_{od}", bufs=2)
                nc.tensor.matmul(op[:], wo_sb[:, od * 128 : (od + 1) * 128], cn[:],
                                 start=True, stop=True)
                nc.vector.tensor_copy(ob2[:, j, :], op[:])
            dst = outp[od0 * 128 : (od0 + 2) * 128, tb + q0 : tb + q0 + 512].rearrange(
                "(h p) c -> p h c", p=128)
            nc.sync.dma_start(dst, ob2[:])

    # round-robin: produce K/V block `blk`, then attention for qi=blk (which
    # needs exactly blocks 0..blk) — keeps dense projection matmuls spread
    # across the whole timeline so the PE clock gate stays open.
    for b in range(B):
        emit_xdma(b, 0)
    nc.sync.dma_start(ident_sb[:], ident[:])
    nc.sync.dma_start(mask_sb[:], mask[:])
    nc.sync.dma_start(wo_sb[:], wo[:])
    for blk in range(NBLK):
        for b in range(B):
            emit_qkv(b, blk)
        if blk + 1 < NBLK:
            for b in range(B):
                emit_xdma(b, blk + 1)
        for b in range(B):
            emit_attn(b, blk)

    ctx.close()


_NC = None


def _get_nc():
    global _NC
    if _NC is None:
        _NC = _build_nc()
    return _NC


def _host_inputs(x, Wq, Wk, Wv, Wo):
    xT = np.ascontiguousarray(x.reshape(NTOK, D).T).astype(np.float32, copy=False)
    tri = (np.arange(128)[:, None] <= np.arange(128)[None, :]).astype(np.float32)
    mask = np.concatenate([np.zeros((128, 128), np.float32), tri], axis=1)
    ident = np.eye(128, dtype=np.float32)
    in_maps = []
    for c in range(NCORES):
        sl = slice(DSL * c, DSL * (c + 1))
        # reference naming: q comes from Wk, k comes from Wq
        wq_c = np.ascontiguousarray(Wk[sl].T) * np.float32(1.0 / np.sqrt(DH))
        wk_c = np.ascontiguousarray(Wq[sl].T)
        wv_c = np.ascontiguousarray(Wv[sl].T)
        woT = np.ascontiguousarray(Wo[:, sl].T)  # [128, 1024]
        in_maps.append({
            "xT": xT, "wq": wq_c, "wk": wk_c, "wv": wv_c, "wo": woT,
            "mask": mask, "ident": ident,
        })
    return in_maps


def kernel(x, Wq, Wk, Wv, Wo, bo, _profile=False):
    x = np.asarray(x, dtype=np.float32)
    nc = _get_nc()
    in_maps = _host_inputs(x, np.asarray(Wq), np.asarray(Wk), np.asarray(Wv), np.asarray(Wo))
    res = run_bass_kernel_spmd(nc, in_maps, core_ids=list(range(NCORES)),
                               trace=bool(_profile))
    acc = np.zeros((D, NTOK), dtype=np.float64)
    for c in range(NCORES):
        acc += res.results[c]["outp"]
    out = acc.T.astype(np.float32) + np.asarray(bo, dtype=np.float32)[None, :]
    if _profile:
        kernel.last_exec_time_ns = res.exec_time_ns
        kernel.last_results = res
    return out.reshape(B, T, D)



# revision 3
# speedup vs baseline: 1.1473x; 1.1473x over previous
"""Multi-head causal attention (B=2, T=2048, D=1024, H=16, Dh=64) on 8 trn2 cores.

Sharding: head-parallel. Core c computes heads (2c, 2c+1) for both batch rows:
  - QKV projections for its 128-dim head slice (bf16 matmuls, K=1024)
  - causal attention for its 2 heads x 2 batches (no max-subtraction softmax;
    scores are O(5) so exp() is safe; 1/sqrt(Dh)=0.125 folded into Wq exactly)
  - partial output projection out_c = ctx_c @ Wo.T[slice]  -> [1024, 4096]
Host sums the 8 partials (fp32), adds bias, reshapes.

All matmuls run in bf16 (fp32 PSUM accumulation): same 1 cycle/row streaming as
fp32r but enables FWL fast weight loads and halves DMA/SBUF. Scores are computed
transposed (ST[tk, tq]) so no P transposes are needed; softmax renormalization
appends 64 replicated ones-columns to V so the denominator Z lands in ctx
partitions 64-127, and 1/Z comes from the DVE reciprocal (no activation-table
swaps). Each block's output projection is deferred into the next block's
score/PV loop so the PE never idles >3.4us (keeps the HAM throttle at K=8/8).
"""

import os
import sys

for _p in ("/opt/trn_rl_repo", "/opt/pypackages",
           "/root/.axon_site/_ro/trn_rl_repo", "/root/.axon_site/_ro/pypackages"):
    if os.path.isdir(_p) and _p not in sys.path:
        sys.path.append(_p)

import numpy as np
import ml_dtypes
import concourse.bass as bass  # noqa: F401  (engine classes referenced via nc)
import concourse.tile as tile
from concourse import bacc, mybir
from concourse.bass_utils import run_bass_kernel_spmd

# NOTE: no --enable-ldw-opt patch here. bf16 matmuls legalize into standalone
# InstLdweights + InstMatmult pairs, which walrus rejects under ldw-opt; the
# PE's reorder queue overlaps weight loads with in-flight matmuls in hardware.

F32 = mybir.dt.float32
BF16 = mybir.dt.bfloat16
AF = mybir.ActivationFunctionType
BF16_NP = ml_dtypes.bfloat16

B, T, D = 2, 2048, 1024
H, DH = 16, 64
NTOK = B * T          # 4096
NCORES = 8
HPC = H // NCORES     # heads per core = 2
DSL = HPC * DH        # per-core d-slice width = 128
KT = D // 128         # contraction tiles = 8
NBLK = T // 512       # tq blocks per batch = 4
NTKT = T // 128       # tk tiles per batch = 16


def _build_nc():
    nc = bacc.Bacc("TRN2", target_bir_lowering=False, debug=False)

    xT = nc.dram_tensor("xT", [D, NTOK], BF16, kind="ExternalInput").ap()
    wq = nc.dram_tensor("wq", [128, KT, 128], BF16, kind="ExternalInput").ap()
    wk = nc.dram_tensor("wk", [128, KT, 128], BF16, kind="ExternalInput").ap()
    wv = nc.dram_tensor("wv", [128, KT, 128], BF16, kind="ExternalInput").ap()
    wo = nc.dram_tensor("wo", [128, KT, 128], BF16, kind="ExternalInput").ap()
    mask = nc.dram_tensor("mask", [128, 256], BF16, kind="ExternalInput").ap()
    ident = nc.dram_tensor("ident", [128, 128], BF16, kind="ExternalInput").ap()
    # blocked output: [p, b*NBLK+qi, od, c] = partial_out[od*128+p, b*2048+qi*512+c]
    outp = nc.dram_tensor("outp", [128, B * NBLK, KT, 512], F32,
                          kind="ExternalOutput").ap()

    with tile.TileContext(nc) as tc:
        _emit(nc, tc, xT, wq, wk, wv, wo, mask, ident, outp)
    nc.compile()
    return nc


def _emit(nc, tc, xT, wq, wk, wv, wo, mask, ident, outp):
    from contextlib import ExitStack

    ctx = ExitStack()
    const = ctx.enter_context(tc.tile_pool(name="const", bufs=1))
    sb = ctx.enter_context(tc.tile_pool(name="sb", bufs=2))
    pt_pool = ctx.enter_context(tc.tile_pool(name="ptp", bufs=4))
    ob_pool = ctx.enter_context(tc.tile_pool(name="obp", bufs=2))
    ps = ctx.enter_context(tc.tile_pool(name="ps", bufs=1, space="PSUM"))

    # ---- constants / persistent SBUF ----
    wq_sb = const.tile([128, KT, 128], BF16)
    wk_sb = const.tile([128, KT, 128], BF16)
    wv_sb = const.tile([128, KT, 128], BF16)
    wo_sb = const.tile([128, KT, 128], BF16)
    mask_sb = const.tile([128, 256], BF16)
    ident_sb = const.tile([128, 128], BF16)
    x_sb = const.tile([128, KT, NTOK], BF16)

    xTr = xT.rearrange("(k p) t -> p k t", p=128)  # [128, 8, 4096]

    qT, kTt, v_ext = {}, {}, {}
    for b in range(B):
        qT[b] = sb.tile([128, T], BF16, tag="qT", name=f"qT{b}")
        kTt[b] = sb.tile([128, T], BF16, tag="kT", name=f"kT{b}")
        v_ext[b] = sb.tile([128, NTKT, 4 * DH], BF16, tag="vext", name=f"vext{b}")
        vons = v_ext[b][:].rearrange("p k (h c) -> p (k h) c", c=2 * DH)[:, :, DH : 2 * DH]
        nc.gpsimd.memset(vons, 1.0)

    # ---- DMA schedule: what the first QKV group needs comes first ----
    nc.sync.dma_start(wq_sb[:], wq)
    for k in range(KT):  # b0 blk0 tokens
        nc.sync.dma_start(x_sb[:, k, 0:512], xTr[:, k, 0:512])
    nc.sync.dma_start(wk_sb[:], wk)
    nc.sync.dma_start(wv_sb[:], wv)
    for k in range(KT):  # b1 blk0 tokens
        nc.sync.dma_start(x_sb[:, k, 2048:2560], xTr[:, k, 2048:2560])
    nc.sync.dma_start(ident_sb[:], ident)
    nc.sync.dma_start(mask_sb[:], mask)
    nc.sync.dma_start(wo_sb[:], wo)
    for k in range(KT):  # b0 rest
        nc.sync.dma_start(x_sb[:, k, 512:2048], xTr[:, k, 512:2048])
    for k in range(KT):  # b1 rest
        nc.sync.dma_start(x_sb[:, k, 2560:4096], xTr[:, k, 2560:4096])

    def emit_qkv(b, blk):
        t0 = b * T + blk * 512
        for wname, w_sb in (("q", wq_sb), ("k", wk_sb), ("v", wv_sb)):
            pp = ps.tile([128, 512], F32, tag="mm", name=f"pp{wname}{b}_{blk}", bufs=2)
            for k in range(KT):
                nc.tensor.matmul(
                    pp[:], w_sb[:, k, :], x_sb[:, k, t0 : t0 + 512],
                    start=(k == 0), stop=(k == KT - 1),
                )
            if wname == "q":
                nc.vector.tensor_copy(qT[b][:, blk * 512 : (blk + 1) * 512], pp[:])
            elif wname == "k":
                nc.vector.tensor_copy(kTt[b][:, blk * 512 : (blk + 1) * 512], pp[:])
            else:
                vst = sb.tile([128, 512], BF16, tag="vst", name=f"vst{b}_{blk}")
                nc.vector.tensor_copy(vst[:], pp[:])
                tr4 = ps.tile([128, 512], BF16, tag="mm", name=f"tr4{b}_{blk}", bufs=2)
                for j in range(4):
                    nc.tensor.transpose(tr4[:, j * 128 : (j + 1) * 128],
                                        vst[:, j * 128 : (j + 1) * 128], ident_sb[:])
                dst = v_ext[b][:, blk * 4 : (blk + 1) * 4, :].rearrange(
                    "p j (h c) -> p j h c", c=2 * DH)[:, :, :, 0:DH]
                nc.vector.tensor_copy(dst, tr4[:].rearrange("p (j h c) -> p j h c", j=4, c=DH))

    def emit_attn(b, qi, deferred):
        """Scores/PV for block (b, qi); runs `deferred` jobs (the previous
        block's output-projection steps) interleaved between tiles. Returns
        this block's deferred jobs."""
        tb = b * T
        q0 = qi * 512
        ntk = 4 * qi + 4
        ctx_pair = ps.tile([128, 2, 512], F32, tag="ctx", name=f"ctx_{b}_{qi}")
        # spread deferred jobs roughly evenly over the tk loop
        sched = {}
        for j, job in enumerate(deferred):
            sched.setdefault(min(ntk - 1, (j * ntk) // max(1, len(deferred))), []).append(job)
        for tk in range(ntk):
            r = tk - 4 * qi
            c0 = 0 if r < 0 else min(128 * r, 256)
            sp = ps.tile([128, 2, 512], F32, tag="s", name=f"sp{b}_{qi}_{tk}", bufs=2)
            for h in range(2):
                hs = slice(h * DH, (h + 1) * DH)
                nc.tensor.matmul(
                    sp[:, h, c0:512],
                    kTt[b][hs, tk * 128 : (tk + 1) * 128],
                    qT[b][hs, q0 + c0 : q0 + 512],
                    start=True, stop=True,
                )
            pt = pt_pool.tile([128, 2, 512], BF16, tag="pt", name=f"pt{b}_{qi}_{tk}")
            nc.scalar.activation(pt[:, :, c0:512], sp[:, :, c0:512], AF.Exp)
            if r >= 0:
                mL = 256 if r == 3 else 128
                msl = mask_sb[:, 256 - mL : 256]
                for h in range(2):
                    seg = pt[:, h, c0 : c0 + mL]
                    nc.vector.tensor_mul(seg, seg, msl)
            for h in range(2):
                nc.tensor.matmul(
                    ctx_pair[:, h, c0:512],
                    v_ext[b][:, tk, h * 2 * DH : (h + 1) * 2 * DH],
                    pt[:, h, c0:512],
                    start=(tk == 0), stop=(tk == ntk - 1),
                )
            for job in sched.get(tk, ()):
                job()
        rz = sb.tile([DH, 2, 512], F32, tag="rz", name=f"rz_{b}_{qi}")
        nc.vector.reciprocal(rz[:], ctx_pair[DH:128, :, :])
        cn = sb.tile([128, 512], BF16, tag="cn", name=f"cn_{b}_{qi}", bufs=2)
        for h in range(2):
            nc.vector.tensor_mul(cn[h * DH : (h + 1) * DH, :],
                                 ctx_pair[0:DH, h, :], rz[:, h, :])
        ob = ob_pool.tile([128, KT, 512], F32, tag="ob", name=f"ob{b}_{qi}")
        bqi = b * NBLK + qi

        def mk_job(od):
            def job():
                op = ps.tile([128, 512], F32, tag="mm", name=f"op{b}_{qi}_{od}", bufs=2)
                nc.tensor.matmul(op[:], wo_sb[:, od, :], cn[:], start=True, stop=True)
                nc.vector.tensor_copy(ob[:, od, :], op[:])
                nc.sync.dma_start(outp[:, bqi, od, :], ob[:, od, :])
            return job

        return [mk_job(od) for od in range(KT)]

    pending = []
    for blk in range(NBLK):
        for b in range(B):
            emit_qkv(b, blk)
        for b in range(B):
            pending = emit_attn(b, blk, pending)
    for job in pending:
        job()

    ctx.close()


_NC = None


def _get_nc():
    global _NC
    if _NC is None:
        _NC = _build_nc()
    return _NC


def _block_weights(w):  # [1024, 128] -> [128, 8, 128] with [p, k, j] = w[k*128+p, j]
    return np.ascontiguousarray(
        w.reshape(KT, 128, 128).transpose(1, 0, 2)).astype(BF16_NP)


def _host_inputs(x, Wq, Wk, Wv, Wo):
    xT = np.ascontiguousarray(x.reshape(NTOK, D).T).astype(BF16_NP)
    tri = (np.arange(128)[:, None] <= np.arange(128)[None, :]).astype(BF16_NP)
    mask = np.concatenate([np.zeros((128, 128), BF16_NP), tri], axis=1)
    ident = np.eye(128, dtype=BF16_NP)
    in_maps = []
    for c in range(NCORES):
        sl = slice(DSL * c, DSL * (c + 1))
        # reference naming: q comes from Wk, k comes from Wq; 1/sqrt(64)=0.125 exact
        wq_c = _block_weights(np.ascontiguousarray(Wk[sl].T) * np.float32(0.125))
        wk_c = _block_weights(np.ascontiguousarray(Wq[sl].T))
        wv_c = _block_weights(np.ascontiguousarray(Wv[sl].T))
        woT = np.ascontiguousarray(Wo[:, sl].T).astype(BF16_NP).reshape(128, KT, 128)
        in_maps.append({
            "xT": xT, "wq": wq_c, "wk": wk_c, "wv": wv_c, "wo": woT,
            "mask": mask, "ident": ident,
        })
    return in_maps


def kernel(x, Wq, Wk, Wv, Wo, bo, _profile=False):
    x = np.asarray(x, dtype=np.float32)
    nc = _get_nc()
    in_maps = _host_inputs(x, np.asarray(Wq), np.asarray(Wk), np.asarray(Wv), np.asarray(Wo))
    res = run_bass_kernel_spmd(nc, in_maps, core_ids=list(range(NCORES)),
                               trace=bool(_profile))
    acc = np.zeros((128, B * NBLK, KT, 512), dtype=np.float64)
    for c in range(NCORES):
        acc += res.results[c]["outp"]
    # [p, bqi, od, c] -> [od*128+p, bqi*512+c]
    full = acc.transpose(2, 0, 1, 3).reshape(D, NTOK)
    out = full.T.astype(np.float32) + np.asarray(bo, dtype=np.float32)[None, :]
    if _profile:
        kernel.last_exec_time_ns = res.exec_time_ns
        kernel.last_results = res
    return out.reshape(B, T, D)


# revision 6
# speedup vs baseline: 1.4029x; 1.2227x over previous
"""Multi-head causal attention (B=2, T=2048, D=1024, H=16, Dh=64) on 8 trn2 cores.

Sharding: head-parallel. Core c computes heads (2c, 2c+1) for both batch rows:
  - QKV projections for its 128-dim head slice (bf16 matmuls, K=1024)
  - causal attention for its 2 heads x 2 batches (no max-subtraction softmax;
    scores are O(5) so exp() is safe; 1/sqrt(Dh)=0.125 folded into Wq exactly)
  - partial output projection out_c = ctx_c @ Wo.T[slice]  -> [1024, 4096]
Host sums the 8 partials (fp32), adds bias, reshapes.

All matmuls run in bf16 (fp32 PSUM accumulation): same 1 cycle/row streaming as
fp32r but enables FWL fast weight loads and halves DMA/SBUF. Scores are computed
transposed (ST[tk, tq]) so no P transposes are needed; softmax renormalization
appends 64 replicated ones-columns to V so the denominator Z lands in ctx
partitions 64-127, and 1/Z comes from the DVE reciprocal (no activation-table
swaps). Each block's output projection is deferred into the next block's
score/PV loop so the PE never idles >3.4us (keeps the HAM throttle at K=8/8).
"""

import os
import sys

for _p in ("/opt/trn_rl_repo", "/opt/pypackages",
           "/root/.axon_site/_ro/trn_rl_repo", "/root/.axon_site/_ro/pypackages"):
    if os.path.isdir(_p) and _p not in sys.path:
        sys.path.append(_p)

import numpy as np
import ml_dtypes
import concourse.bass as bass  # noqa: F401  (engine classes referenced via nc)
import concourse.tile as tile
from concourse import bacc, mybir
from concourse.bass_utils import run_bass_kernel_spmd

# NOTE: no --enable-ldw-opt patch here. bf16 matmuls legalize into standalone
# InstLdweights + InstMatmult pairs, which walrus rejects under ldw-opt; the
# PE's reorder queue overlaps weight loads with in-flight matmuls in hardware.

F32 = mybir.dt.float32
BF16 = mybir.dt.bfloat16
AF = mybir.ActivationFunctionType
BF16_NP = ml_dtypes.bfloat16

B, T, D = 2, 2048, 1024
H, DH = 16, 64
NTOK = B * T          # 4096
NCORES = 8
HPC = H // NCORES     # heads per core = 2
DSL = HPC * DH        # per-core d-slice width = 128
KT = D // 128         # contraction tiles = 8
NBLK = T // 512       # tq blocks per batch = 4
NTKT = T // 128       # tk tiles per batch = 16


def _build_nc():
    nc = bacc.Bacc("TRN2", target_bir_lowering=False, debug=False)

    xT = nc.dram_tensor("xT", [D, NTOK], BF16, kind="ExternalInput").ap()
    wq = nc.dram_tensor("wq", [128, KT, 128], BF16, kind="ExternalInput").ap()
    wk = nc.dram_tensor("wk", [128, KT, 128], BF16, kind="ExternalInput").ap()
    wv = nc.dram_tensor("wv", [128, KT, 128], BF16, kind="ExternalInput").ap()
    wo = nc.dram_tensor("wo", [128, KT, 128], BF16, kind="ExternalInput").ap()
    mask = nc.dram_tensor("mask", [128, 256], BF16, kind="ExternalInput").ap()
    ident = nc.dram_tensor("ident", [128, 128], BF16, kind="ExternalInput").ap()
    # blocked output: [p, b*NBLK+qi, od, c] = partial_out[od*128+p, b*2048+qi*512+c]
    outp = nc.dram_tensor("outp", [128, B * NBLK, KT, 512], F32,
                          kind="ExternalOutput").ap()

    with tile.TileContext(nc) as tc:
        _emit(nc, tc, xT, wq, wk, wv, wo, mask, ident, outp)
    nc.compile()
    return nc


def _emit(nc, tc, xT, wq, wk, wv, wo, mask, ident, outp):
    from contextlib import ExitStack

    ctx = ExitStack()
    const = ctx.enter_context(tc.tile_pool(name="const", bufs=1))
    sb = ctx.enter_context(tc.tile_pool(name="sb", bufs=2))
    pt_pool = ctx.enter_context(tc.tile_pool(name="ptp", bufs=4))
    ob_pool = ctx.enter_context(tc.tile_pool(name="obp", bufs=2))
    ps = ctx.enter_context(tc.tile_pool(name="ps", bufs=1, space="PSUM"))

    # ---- constants / persistent SBUF ----
    wq_sb = const.tile([128, KT, 128], BF16)
    wk_sb = const.tile([128, KT, 128], BF16)
    wv_sb = const.tile([128, KT, 128], BF16)
    wo_sb = const.tile([128, KT, 128], BF16)
    mask_sb = const.tile([128, 256], BF16)
    ident_sb = const.tile([128, 128], BF16)
    x_sb = const.tile([128, KT, NTOK], BF16)

    xTr = xT.rearrange("(k p) t -> p k t", p=128)  # [128, 8, 4096]

    qT, kTt, v_ext = {}, {}, {}
    for b in range(B):
        qT[b] = sb.tile([128, T], BF16, tag="qT", name=f"qT{b}")
        kTt[b] = sb.tile([128, T], BF16, tag="kT", name=f"kT{b}")
        v_ext[b] = sb.tile([128, NTKT, 4 * DH], BF16, tag="vext", name=f"vext{b}")
        vons = v_ext[b][:].rearrange("p k (h c) -> p (k h) c", c=2 * DH)[:, :, DH : 2 * DH]
        nc.gpsimd.memset(vons, 1.0)

    # ---- DMA schedule: what the first QKV group needs comes first ----
    nc.sync.dma_start(wq_sb[:], wq)
    for k in range(KT):  # b0 blk0 tokens
        nc.sync.dma_start(x_sb[:, k, 0:512], xTr[:, k, 0:512])
    nc.sync.dma_start(wk_sb[:], wk)
    nc.sync.dma_start(wv_sb[:], wv)
    for k in range(KT):  # b1 blk0 tokens
        nc.sync.dma_start(x_sb[:, k, 2048:2560], xTr[:, k, 2048:2560])
    nc.sync.dma_start(ident_sb[:], ident)
    nc.sync.dma_start(mask_sb[:], mask)
    nc.sync.dma_start(wo_sb[:], wo)
    for k in range(KT):  # b0 rest
        nc.sync.dma_start(x_sb[:, k, 512:2048], xTr[:, k, 512:2048])
    for k in range(KT):  # b1 rest
        nc.sync.dma_start(x_sb[:, k, 2560:4096], xTr[:, k, 2560:4096])

    def emit_qkv(b, blk):
        t0 = b * T + blk * 512
        for wname, w_sb in (("q", wq_sb), ("k", wk_sb), ("v", wv_sb)):
            pp = ps.tile([128, 512], F32, tag="mm", name=f"pp{wname}{b}_{blk}", bufs=2)
            for k in range(KT):
                nc.tensor.matmul(
                    pp[:], w_sb[:, k, :], x_sb[:, k, t0 : t0 + 512],
                    start=(k == 0), stop=(k == KT - 1),
                )
            if wname == "q":
                nc.vector.tensor_copy(qT[b][:, blk * 512 : (blk + 1) * 512], pp[:])
            elif wname == "k":
                nc.vector.tensor_copy(kTt[b][:, blk * 512 : (blk + 1) * 512], pp[:])
            else:
                vst = sb.tile([128, 512], BF16, tag="vst", name=f"vst{b}_{blk}")
                nc.vector.tensor_copy(vst[:], pp[:])
                tr4 = ps.tile([128, 512], BF16, tag="mm", name=f"tr4{b}_{blk}", bufs=2)
                for j in range(4):
                    nc.tensor.transpose(tr4[:, j * 128 : (j + 1) * 128],
                                        vst[:, j * 128 : (j + 1) * 128], ident_sb[:])
                dst = v_ext[b][:, blk * 4 : (blk + 1) * 4, :].rearrange(
                    "p j (h c) -> p j h c", c=2 * DH)[:, :, :, 0:DH]
                nc.vector.tensor_copy(dst, tr4[:].rearrange("p (j h c) -> p j h c", j=4, c=DH))

    def emit_attn(b, qi, deferred):
        """Scores/PV for block (b, qi); runs `deferred` jobs (the previous
        block's output-projection steps) interleaved between tiles. Returns
        this block's deferred jobs."""
        tb = b * T
        q0 = qi * 512
        ntk = 4 * qi + 4
        ctx_pair = ps.tile([128, 2, 512], F32, tag="ctx", name=f"ctx_{b}_{qi}")
        # spread deferred jobs roughly evenly over the tk loop, starting at
        # tk=1 so the previous block's 1/Z + normalize has a tile of slack
        sched = {}
        for j, job in enumerate(deferred):
            slot = 1 + (j * (ntk - 1)) // max(1, len(deferred))
            sched.setdefault(min(ntk - 1, slot), []).append(job)
        for tk in range(ntk):
            r = tk - 4 * qi
            c0 = 0 if r < 0 else min(128 * r, 256)
            sp = ps.tile([128, 2, 512], F32, tag="s", name=f"sp{b}_{qi}_{tk}", bufs=2)
            for h in range(2):
                hs = slice(h * DH, (h + 1) * DH)
                nc.tensor.matmul(
                    sp[:, h, c0:512],
                    kTt[b][hs, tk * 128 : (tk + 1) * 128],
                    qT[b][hs, q0 + c0 : q0 + 512],
                    start=True, stop=True,
                )
            pt = pt_pool.tile([128, 2, 512], BF16, tag="pt", name=f"pt{b}_{qi}_{tk}")
            nc.scalar.activation(pt[:, :, c0:512], sp[:, :, c0:512], AF.Exp)
            if r >= 0:
                mL = 256 if r == 3 else 128
                msl = mask_sb[:, 256 - mL : 256]
                for h in range(2):
                    seg = pt[:, h, c0 : c0 + mL]
                    nc.vector.tensor_mul(seg, seg, msl)
            # independent deferred work goes between S and PV: the PE executes
            # in order and PV waits on exp/mask, so these fill that latency
            for job in sched.get(tk, ()):
                job()
            for h in range(2):
                nc.tensor.matmul(
                    ctx_pair[:, h, c0:512],
                    v_ext[b][:, tk, h * 2 * DH : (h + 1) * 2 * DH],
                    pt[:, h, c0:512],
                    start=(tk == 0), stop=(tk == ntk - 1),
                )
        # stage Z into SBUF at partition base 0 (custom DVE op misbehaves on
        # PSUM / offset-partition sources), then fast approximate 1/Z
        zb = sb.tile([DH, 2, 512], F32, tag="zb", name=f"zb_{b}_{qi}")
        nc.vector.tensor_copy(zb[:], ctx_pair[DH:128, :, :])
        rz = sb.tile([DH, 2, 512], F32, tag="rz", name=f"rz_{b}_{qi}")
        nc.vector.reciprocal_approx_fast(rz[:], zb[:])
        cn = sb.tile([128, 512], BF16, tag="cn", name=f"cn_{b}_{qi}", bufs=2)
        for h in range(2):
            nc.vector.tensor_mul(cn[h * DH : (h + 1) * DH, :],
                                 ctx_pair[0:DH, h, :], rz[:, h, :])
        ob = ob_pool.tile([128, KT, 512], F32, tag="ob", name=f"ob{b}_{qi}")
        bqi = b * NBLK + qi

        def mk_job(od):
            def job():
                op = ps.tile([128, 512], F32, tag="mm", name=f"op{b}_{qi}_{od}", bufs=2)
                nc.tensor.matmul(op[:], wo_sb[:, od, :], cn[:], start=True, stop=True)
                nc.vector.tensor_copy(ob[:, od, :], op[:])
                nc.sync.dma_start(outp[:, bqi, od, :], ob[:, od, :])
            return job

        return [mk_job(od) for od in range(KT)]

    pending = []
    for blk in range(NBLK):
        for b in range(B):
            emit_qkv(b, blk)
        for b in range(B):
            pending = emit_attn(b, blk, pending)
    for job in pending:
        job()

    ctx.close()


_NC = None


def _get_nc():
    global _NC
    if _NC is None:
        _NC = _build_nc()
    return _NC


def _block_weights(w):  # [1024, 128] -> [128, 8, 128] with [p, k, j] = w[k*128+p, j]
    return np.ascontiguousarray(
        w.reshape(KT, 128, 128).transpose(1, 0, 2)).astype(BF16_NP)


def _host_inputs(x, Wq, Wk, Wv, Wo):
    xT = np.ascontiguousarray(x.reshape(NTOK, D).T).astype(BF16_NP)
    tri = (np.arange(128)[:, None] <= np.arange(128)[None, :]).astype(BF16_NP)
    mask = np.concatenate([np.zeros((128, 128), BF16_NP), tri], axis=1)
    ident = np.eye(128, dtype=BF16_NP)
    in_maps = []
    for c in range(NCORES):
        sl = slice(DSL * c, DSL * (c + 1))
        # reference naming: q comes from Wk, k comes from Wq; 1/sqrt(64)=0.125 exact
        wq_c = _block_weights(np.ascontiguousarray(Wk[sl].T) * np.float32(0.125))
        wk_c = _block_weights(np.ascontiguousarray(Wq[sl].T))
        wv_c = _block_weights(np.ascontiguousarray(Wv[sl].T))
        woT = np.ascontiguousarray(Wo[:, sl].T).astype(BF16_NP).reshape(128, KT, 128)
        in_maps.append({
            "xT": xT, "wq": wq_c, "wk": wk_c, "wv": wv_c, "wo": woT,
            "mask": mask, "ident": ident,
        })
    return in_maps


def kernel(x, Wq, Wk, Wv, Wo, bo, _profile=False):
    x = np.asarray(x, dtype=np.float32)
    nc = _get_nc()
    in_maps = _host_inputs(x, np.asarray(Wq), np.asarray(Wk), np.asarray(Wv), np.asarray(Wo))
    res = run_bass_kernel_spmd(nc, in_maps, core_ids=list(range(NCORES)),
                               trace=bool(_profile))
    acc = np.zeros((128, B * NBLK, KT, 512), dtype=np.float64)
    for c in range(NCORES):
        acc += res.results[c]["outp"]
    # [p, bqi, od, c] -> [od*128+p, bqi*512+c]
    full = acc.transpose(2, 0, 1, 3).reshape(D, NTOK)
    out = full.T.astype(np.float32) + np.asarray(bo, dtype=np.float32)[None, :]
    if _profile:
        kernel.last_exec_time_ns = res.exec_time_ns
        kernel.last_results = res
    return out.reshape(B, T, D)


# revision 11
# speedup vs baseline: 1.4667x; 1.0455x over previous
"""Multi-head causal attention (B=2, T=2048, D=1024, H=16, Dh=64) on 8 trn2 cores.

Sharding: head-parallel. Core c computes heads (2c, 2c+1) for both batch rows:
  - QKV projections for its 128-dim head slice (bf16 matmuls, K=1024)
  - causal attention for its 2 heads x 2 batches (no max-subtraction softmax;
    scores are O(5) so exp() is safe; 1/sqrt(Dh)=0.125 folded into Wq exactly)
  - partial output projection out_c = ctx_c @ Wo.T[slice]  -> [1024, 4096]
Host sums the 8 partials (fp32), adds bias, reshapes.

All matmuls run in bf16 (fp32 PSUM accumulation): same 1 cycle/row streaming as
fp32r but enables FWL fast weight loads and halves DMA/SBUF. Scores are computed
transposed (ST[tk, tq]) so no P transposes are needed; softmax renormalization
appends 64 replicated ones-columns to V so the denominator Z lands in ctx
partitions 64-127, and 1/Z comes from the DVE reciprocal (no activation-table
swaps). Each block's output projection is deferred into the next block's
score/PV loop so the PE never idles >3.4us (keeps the HAM throttle at K=8/8).
"""

import os
import sys

for _p in ("/opt/trn_rl_repo", "/opt/pypackages",
           "/root/.axon_site/_ro/trn_rl_repo", "/root/.axon_site/_ro/pypackages"):
    if os.path.isdir(_p) and _p not in sys.path:
        sys.path.append(_p)

import numpy as np
import ml_dtypes
import concourse.bass as bass  # noqa: F401  (engine classes referenced via nc)
import concourse.tile as tile
from concourse import bacc, mybir
from concourse.bass_utils import run_bass_kernel_spmd

# NOTE: no --enable-ldw-opt patch here. bf16 matmuls legalize into standalone
# InstLdweights + InstMatmult pairs, which walrus rejects under ldw-opt; the
# PE's reorder queue overlaps weight loads with in-flight matmuls in hardware.

F32 = mybir.dt.float32
BF16 = mybir.dt.bfloat16
AF = mybir.ActivationFunctionType
BF16_NP = ml_dtypes.bfloat16

B, T, D = 2, 2048, 1024
H, DH = 16, 64
NTOK = B * T          # 4096
NCORES = 8
HPC = H // NCORES     # heads per core = 2
DSL = HPC * DH        # per-core d-slice width = 128
KT = D // 128         # contraction tiles = 8
NBLK = T // 512       # tq blocks per batch = 4
NTKT = T // 128       # tk tiles per batch = 16


def _build_nc():
    nc = bacc.Bacc("TRN2", target_bir_lowering=False, debug=False)

    xT = nc.dram_tensor("xT", [D, NTOK], BF16, kind="ExternalInput").ap()
    wq = nc.dram_tensor("wq", [128, KT, 128], BF16, kind="ExternalInput").ap()
    wk = nc.dram_tensor("wk", [128, KT, 128], BF16, kind="ExternalInput").ap()
    wv = nc.dram_tensor("wv", [128, KT, 128], BF16, kind="ExternalInput").ap()
    wo = nc.dram_tensor("wo", [128, KT, 128], BF16, kind="ExternalInput").ap()
    mask = nc.dram_tensor("mask", [128, 256], BF16, kind="ExternalInput").ap()
    ident = nc.dram_tensor("ident", [128, 128], BF16, kind="ExternalInput").ap()
    # blocked output: [p, b*NBLK+qi, od, c] = partial_out[od*128+p, b*2048+qi*512+c]
    outp = nc.dram_tensor("outp", [128, B * NBLK, KT, 512], F32,
                          kind="ExternalOutput").ap()

    with tile.TileContext(nc) as tc:
        _emit(nc, tc, xT, wq, wk, wv, wo, mask, ident, outp)
    nc.compile()
    return nc


def _emit(nc, tc, xT, wq, wk, wv, wo, mask, ident, outp):
    from contextlib import ExitStack

    ctx = ExitStack()
    const = ctx.enter_context(tc.tile_pool(name="const", bufs=1))
    sb = ctx.enter_context(tc.tile_pool(name="sb", bufs=2))
    pt_pool = ctx.enter_context(tc.tile_pool(name="ptp", bufs=4))
    ob_pool = ctx.enter_context(tc.tile_pool(name="obp", bufs=2))
    ps = ctx.enter_context(tc.tile_pool(name="ps", bufs=1, space="PSUM"))

    # ---- constants / persistent SBUF ----
    wq_sb = const.tile([128, KT, 128], BF16)
    wk_sb = const.tile([128, KT, 128], BF16)
    wv_sb = const.tile([128, KT, 128], BF16)
    wo_sb = const.tile([128, KT, 128], BF16)
    mask_sb = const.tile([128, 256], BF16)
    ident_sb = const.tile([128, 128], BF16)
    x_sb = const.tile([128, KT, NTOK], BF16)

    xTr = xT.rearrange("(k p) t -> p k t", p=128)  # [128, 8, 4096]

    qT, kTt, v_ext = {}, {}, {}
    for b in range(B):
        qT[b] = sb.tile([128, T], BF16, tag="qT", name=f"qT{b}")
        kTt[b] = sb.tile([128, T], BF16, tag="kT", name=f"kT{b}")
        v_ext[b] = sb.tile([128, NTKT, 4 * DH], BF16, tag="vext", name=f"vext{b}")
        vons = v_ext[b][:].rearrange("p k (h c) -> p (k h) c", c=2 * DH)[:, :, DH : 2 * DH]
        nc.gpsimd.memset(vons, 1.0)

    # ---- DMA schedule: what the first QKV group needs comes first, at the
    # finest useful granularity (mm k waits only on wq[:,k,:] + x[:,k,blk0]) --
    for k in range(KT):
        nc.sync.dma_start(wq_sb[:, k, :], wq[:, k, :])
        nc.sync.dma_start(x_sb[:, k, 0:512], xTr[:, k, 0:512])
    nc.sync.dma_start(wk_sb[:], wk)
    nc.sync.dma_start(wv_sb[:], wv)
    for k in range(KT):  # b1 blk0 tokens
        nc.sync.dma_start(x_sb[:, k, 2048:2560], xTr[:, k, 2048:2560])
    nc.sync.dma_start(ident_sb[:], ident)
    nc.sync.dma_start(mask_sb[:], mask)
    nc.sync.dma_start(wo_sb[:], wo)
    for k in range(KT):  # b0 rest
        nc.sync.dma_start(x_sb[:, k, 512:2048], xTr[:, k, 512:2048])
    for k in range(KT):  # b1 rest
        nc.sync.dma_start(x_sb[:, k, 2560:4096], xTr[:, k, 2560:4096])

    vst_t = {}

    def qkv_proj(b, blk, wname, w_sb):
        t0 = b * T + blk * 512
        pp = ps.tile([128, 512], F32, tag="mm", name=f"pp{wname}{b}_{blk}", bufs=2)
        for k in range(KT):
            nc.tensor.matmul(
                pp[:], w_sb[:, k, :], x_sb[:, k, t0 : t0 + 512],
                start=(k == 0), stop=(k == KT - 1),
            )
        if wname == "q":
            nc.vector.tensor_copy(qT[b][:, blk * 512 : (blk + 1) * 512], pp[:])
        elif wname == "k":
            nc.vector.tensor_copy(kTt[b][:, blk * 512 : (blk + 1) * 512], pp[:])
        else:
            vst = sb.tile([128, 512], BF16, tag="vst", name=f"vst{b}_{blk}")
            nc.vector.tensor_copy(vst[:], pp[:])
            vst_t[(b, blk)] = vst

    def v_tail(b, blk):
        # transposes + v_ext scatter; separate job so the PE-side transposes
        # don't sit waiting on the vst cast inside one job
        vst = vst_t.pop((b, blk))
        tr4 = ps.tile([128, 512], BF16, tag="mm", name=f"tr4{b}_{blk}", bufs=2)
        for j in range(4):
            nc.tensor.transpose(tr4[:, j * 128 : (j + 1) * 128],
                                vst[:, j * 128 : (j + 1) * 128], ident_sb[:])
        dst = v_ext[b][:, blk * 4 : (blk + 1) * 4, :].rearrange(
            "p j (h c) -> p j h c", c=2 * DH)[:, :, :, 0:DH]
        nc.vector.tensor_copy(dst, tr4[:].rearrange("p (j h c) -> p j h c", j=4, c=DH))

    def emit_qkv(b, blk):
        for wname, w_sb in (("q", wq_sb), ("k", wk_sb), ("v", wv_sb)):
            qkv_proj(b, blk, wname, w_sb)
            if wname == "v":
                v_tail(b, blk)

    def qkv_jobs(b, blk):
        """QKV projections for (b, blk) as deferred jobs (min_slot 0)."""
        jobs = [
            (lambda: qkv_proj(b, blk, "q", wq_sb), 0),
            (lambda: qkv_proj(b, blk, "k", wk_sb), 0),
            (lambda: qkv_proj(b, blk, "v", wv_sb), 0),
            (lambda: v_tail(b, blk), 0),
        ]
        return jobs

    def emit_attn(b, qi, deferred):
        """Scores/PV for block (b, qi); runs `deferred` (job, min_slot) pairs
        (the previous block's output projection + the next round's QKV)
        interleaved between tiles. Returns this block's deferred jobs."""
        tb = b * T
        q0 = qi * 512
        ntk = 4 * qi + 4
        ctx_pair = ps.tile([128, 2, 512], F32, tag="ctx", name=f"ctx_{b}_{qi}")
        # spread deferred jobs roughly evenly over the tk loop, honoring each
        # job's earliest allowed slot
        sched = {}
        nj = max(1, len(deferred))
        for j, (job, ms) in enumerate(deferred):
            slot = max(ms, min(ntk - 1, (j * ntk) // nj))
            sched.setdefault(min(ntk - 1, slot), []).append(job)
        for tk in range(ntk):
            r = tk - 4 * qi
            c0 = 0 if r < 0 else min(128 * r, 256)
            sp = ps.tile([128, 2, 512], F32, tag="s", name=f"sp{b}_{qi}_{tk}", bufs=2)
            for h in range(2):
                hs = slice(h * DH, (h + 1) * DH)
                nc.tensor.matmul(
                    sp[:, h, c0:512],
                    kTt[b][hs, tk * 128 : (tk + 1) * 128],
                    qT[b][hs, q0 + c0 : q0 + 512],
                    start=True, stop=True,
                )
            pt = pt_pool.tile([128, 2, 512], BF16, tag="pt", name=f"pt{b}_{qi}_{tk}")
            nc.scalar.activation(pt[:, :, c0:512], sp[:, :, c0:512], AF.Exp)
            if r >= 0:
                mL = 256 if r == 3 else 128
                msl = mask_sb[:, 256 - mL : 256]
                for h in range(2):
                    seg = pt[:, h, c0 : c0 + mL]
                    nc.vector.tensor_mul(seg, seg, msl)
            # independent deferred work goes between S and PV: the PE executes
            # in order and PV waits on exp/mask, so these fill that latency
            for job in sched.get(tk, ()):
                job()
            for h in range(2):
                nc.tensor.matmul(
                    ctx_pair[:, h, c0:512],
                    v_ext[b][:, tk, h * 2 * DH : (h + 1) * 2 * DH],
                    pt[:, h, c0:512],
                    start=(tk == 0), stop=(tk == ntk - 1),
                )
        # stage Z into SBUF at partition base 0 (custom DVE op misbehaves on
        # PSUM / offset-partition sources), then fast approximate 1/Z
        zb = sb.tile([DH, 2, 512], F32, tag="zb", name=f"zb_{b}_{qi}")
        nc.vector.tensor_copy(zb[:], ctx_pair[DH:128, :, :])
        rz = sb.tile([DH, 2, 512], F32, tag="rz", name=f"rz_{b}_{qi}")
        nc.vector.reciprocal_approx_fast(rz[:], zb[:])
        cn = sb.tile([128, 512], BF16, tag="cn", name=f"cn_{b}_{qi}", bufs=2)
        for h in range(2):
            nc.vector.tensor_mul(cn[h * DH : (h + 1) * DH, :],
                                 ctx_pair[0:DH, h, :], rz[:, h, :])
        ob = ob_pool.tile([128, KT, 512], F32, tag="ob", name=f"ob{b}_{qi}")
        bqi = b * NBLK + qi

        def mk_job(od, copy_eng="v"):
            def job():
                op = ps.tile([128, 512], F32, tag="mm", name=f"op{b}_{qi}_{od}", bufs=2)
                nc.tensor.matmul(op[:], wo_sb[:, od, :], cn[:], start=True, stop=True)
                if copy_eng == "s":
                    nc.scalar.copy(ob[:, od, :], op[:])
                else:
                    nc.vector.tensor_copy(ob[:, od, :], op[:])
                nc.sync.dma_start(outp[:, bqi, od, :], ob[:, od, :])
            return job

        final = (b, qi) == (B - 1, NBLK - 1)
        # final block: alternate copy engines so the tail drains ~2x faster
        return [(mk_job(od, "s" if final and od % 2 else "v"), 1) for od in range(KT)]

    # round 0 QKV is emitted directly; later rounds' QKV interleaves into the
    # previous round's attention tile loops as deferred jobs
    for b in range(B):
        emit_qkv(b, 0)
    pending = []
    for blk in range(NBLK):
        for b in range(B):
            nxt = qkv_jobs(b, blk + 1) if blk + 1 < NBLK else []
            pending = emit_attn(b, blk, nxt + pending)
    for job, _ in pending:
        job()

    ctx.close()


_NC = None


def _get_nc():
    global _NC
    if _NC is None:
        _NC = _build_nc()
    return _NC


def _block_weights(w):  # [1024, 128] -> [128, 8, 128] with [p, k, j] = w[k*128+p, j]
    return np.ascontiguousarray(
        w.reshape(KT, 128, 128).transpose(1, 0, 2)).astype(BF16_NP)


def _host_inputs(x, Wq, Wk, Wv, Wo):
    xT = np.ascontiguousarray(x.reshape(NTOK, D).T).astype(BF16_NP)
    tri = (np.arange(128)[:, None] <= np.arange(128)[None, :]).astype(BF16_NP)
    mask = np.concatenate([np.zeros((128, 128), BF16_NP), tri], axis=1)
    ident = np.eye(128, dtype=BF16_NP)
    in_maps = []
    for c in range(NCORES):
        sl = slice(DSL * c, DSL * (c + 1))
        # reference naming: q comes from Wk, k comes from Wq; 1/sqrt(64)=0.125 exact
        wq_c = _block_weights(np.ascontiguousarray(Wk[sl].T) * np.float32(0.125))
        wk_c = _block_weights(np.ascontiguousarray(Wq[sl].T))
        wv_c = _block_weights(np.ascontiguousarray(Wv[sl].T))
        woT = np.ascontiguousarray(Wo[:, sl].T).astype(BF16_NP).reshape(128, KT, 128)
        in_maps.append({
            "xT": xT, "wq": wq_c, "wk": wk_c, "wv": wv_c, "wo": woT,
            "mask": mask, "ident": ident,
        })
    return in_maps


def kernel(x, Wq, Wk, Wv, Wo, bo, _profile=False):
    x = np.asarray(x, dtype=np.float32)
    nc = _get_nc()
    in_maps = _host_inputs(x, np.asarray(Wq), np.asarray(Wk), np.asarray(Wv), np.asarray(Wo))
    res = run_bass_kernel_spmd(nc, in_maps, core_ids=list(range(NCORES)),
                               trace=bool(_profile))
    acc = np.zeros((128, B * NBLK, KT, 512), dtype=np.float64)
    for c in range(NCORES):
        acc += res.results[c]["outp"]
    # [p, bqi, od, c] -> [od*128+p, bqi*512+c]
    full = acc.transpose(2, 0, 1, 3).reshape(D, NTOK)
    out = full.T.astype(np.float32) + np.asarray(bo, dtype=np.float32)[None, :]
    if _profile:
        kernel.last_exec_time_ns = res.exec_time_ns
        kernel.last_results = res
    return out.reshape(B, T, D)
